# revision 1
# baseline (speedup 1.0000x reference)
"""Trainium2 Bass kernel for DendriticFullyConnected.

Math (B=128, IN=OUT=1024):
    state = sigmoid(x @ W_non.T + b_non) - 1
    syn   = x[:,None,:] * W_nmda[None,:,:]            # [B,O,I]
    clus  = 0.5*(syn[...,:-1] + syn[...,1:])          # conv [0.5,0.5]
    ca    = relu(clus.sum(-1))
    n     = 2 + state;  kd = 0.5**n;  xn = ca**n
    out   = xn/(kd+xn) + state

Key algebraic collapse: the conv+sum equals a plain dot product against
W_nmda with its first and last columns halved:
    clus.sum(-1)[b,o] = sum_i x[b,i]*Wm[o,i],  Wm = W_nmda w/ cols {0,-1} * 0.5
so the [B,O,I] tensor never exists - the whole module is two matmuls:
    z_non = x @ W_non.T + b_non ;  z_nmda = x @ Wm.T
and the Hill fraction is a sigmoid in log space:
    xn/(kd+xn) = sigmoid(n * (ln(ca) + ln 2))

Sharding: OUT split 8 ways (128 outputs/core), x replicated. Per-core HBM
traffic ~1.6MB; every weight byte is read exactly once across the chip.

Device-side design:
- Host prepares all layouts; zero on-device transposes. Contraction index
  lands on SBUF partitions for both operands.
- W_non/Wm are interleaved per K-chunk into one [128, 2048] stream so both
  z's accumulate in a single PSUM tile per matmul (N=256, one pass over x).
- Matmul inputs are bf16 (f32 PSUM accumulate): the kernel is memory-bound
  and this halves HBM traffic; measured output rel-err 2.0e-3 against the
  f32 reference (family pass gate is rel_err < 2e-2). The b_non bias is
  applied as two bf16 K-rows (hi+lo split, exact to ~2^-17) via
  lhsT=ones, rhs=[b_hi|0],[b_lo|0].
- Dummy matmuls at t~0 warm the PE HAM clock gate to full rate before
  the real matmuls arrive.
- The activation-table pass is overridden to use the single
  natural_log_exp_and_others set (exp+ln+relu); default behavior loads a
  different table set per function and ping-pongs 4 table loads (~2.7us
  each pair) through the critical path. A dummy Ln at t=0 pulls the one
  remaining load under the DMA shadow.
"""

import numpy as np

_B, _IN, _OUT, _NC = 128, 1024, 1024, 8
_OSH = _OUT // _NC  # 128 outputs per core
_KT = _IN // 128    # 8 contraction chunks
_XCH = 1            # DMA chunks for the x stream
_WCH = 2            # DMA chunks for the interleaved weight stream
_WSPLIT = [5, 3]    # weight chunk sizes in K-chunks (uneven: small tail)
_MMDT = "bfloat16"  # matmul input dtype (bfloat16 | float32 | float32r)

_PIN_ACT_SET = "natural_log_exp_and_others"
_NWARM = 7          # PE warmup matmuls

_state = {}


def _rearr(m):
    # [128 rows, 1024 cols] -> out[p, j*128 + r] = m[r, j*128 + p]
    # per 128-column chunk j: transpose so the contraction index is the
    # partition dim and the row index is the free dim.
    return np.ascontiguousarray(
        m.reshape(128, _KT, 128).transpose(2, 1, 0).reshape(128, _IN)
    )


def _make_bacc_cls():
    import concourse.bacc as bacc
    import concourse.mybir as mybir
    from concourse.hw_specs import get_activation_tables
    import bass_rust as _bass_rust

    class PinnedActBacc(bacc.Bacc):
        """Force all activations onto one table set so the kernel pays a
        single ACT table load instead of one per ln<->exp transition."""

        def insert_act_table_loads(self):
            has_activation = any(
                isinstance(i, mybir.InstActivation)
                for b in self.main_func.blocks
                for i in b.instructions
            )
            if not has_activation:
                return
            tables = list(get_activation_tables(self.m.arch).items())
            names = [t[0] for t in tables]
            if _PIN_ACT_SET not in names:
                _bass_rust.insert_act_table_loads(self, tables)
                return
            canon = names.index(_PIN_ACT_SET)
            keep = [tables[canon]]
            _bass_rust.insert_act_table_loads(self, keep)
            # the pass writes positional ids into the filtered list; remap
            # to the canonical act_info.json index walrus expects.
            for b in self.main_func.blocks:
                for i in b.instructions:
                    if isinstance(i, mybir.InstLoadActFuncSet):
                        i.act_func_set_id = canon

    return PinnedActBacc


def _build(loop_n=None, mm_dtype=None, xch=None, wch=None, wsplit=None):
    mm_dtype = mm_dtype or _MMDT
    xch = xch or _XCH
    wch = wch or _WCH
    wsplit = wsplit or _WSPLIT
    if wsplit is not None:
        wch = len(wsplit)
    import concourse.mybir as mybir
    import concourse.tile as tile
    from concourse.bass import ts
    from concourse.bass_utils import run_bass_kernel_spmd

    dt = mybir.dt.float32
    mdt = getattr(mybir.dt, mm_dtype)
    AF = mybir.ActivationFunctionType
    OP = mybir.AluOpType

    nc = _make_bacc_cls()(
        "TRN2",
        target_bir_lowering=False,
        debug=False,
        enable_asserts=False,
        num_devices=_NC,
    )
    xT = nc.dram_tensor("xT", [128, _IN], mdt, kind="ExternalInput").ap()
    wc = nc.dram_tensor("wc", [128, 2 * _IN], mdt, kind="ExternalInput").ap()
    bc = nc.dram_tensor("bc", [2, 2 * _OSH], mdt, kind="ExternalInput").ap()
    out = nc.dram_tensor("out", [_B, _OSH], dt, kind="ExternalOutput").ap()

    XW = _IN // xch      # x cols per DMA chunk
    WW = 2 * _IN // wch  # wcat cols per DMA chunk
    XJ = _KT // xch      # K-chunks per x DMA chunk
    WJ = _KT // wch      # K-chunks per wcat DMA chunk

    def body(tc, io, ep, ps):
        # PE warmup: dummy matmuls starting at t~0 lift the HAM clock gate
        # to full rate before the real matmuls arrive; sized to end right
        # as the first weight chunk lands.
        wsrc = io.tile([2, 512], mybir.dt.bfloat16)
        nc.vector.memset(wsrc[:], 0.0)

        # ACT table warm: pulls the single natural_log_exp set load to t=0.
        warm0 = ep.tile([1, 1], dt)
        nc.vector.memset(warm0[:], 1.0)
        warm1 = ep.tile([1, 1], dt)
        nc.scalar.activation(warm1[:], warm0[:], AF.Ln)

        ones = io.tile([2, _B], mybir.dt.bfloat16)
        nc.vector.memset(ones[:], 1.0)
        # eps doubles as the u >= -42.5 clamp: ln(3.36e-19) = -42.5, which
        # keeps t = n*u >= -85 so exp(-t) stays finite (n < 2)
        eps = io.tile([128, 1], dt)
        nc.vector.memset(eps[:], 3.36e-19)

        wp = ps.tile([128, 512], dt)
        for k in range(_NWARM):
            nc.tensor.matmul(
                wp[:], wsrc[:, 0:128], wsrc[:],
                start=(k == 0), stop=(k == _NWARM - 1),
            )

        # chunked loads; issue order = rough priority order (first matmul
        # chunks as early as possible, tiny bias row squeezed in between)
        # weight stream split unevenly: big chunk first, small last chunk so
        # the final matmuls are gated on as little trailing DMA as possible
        WSPLIT = wsplit or ([6, 2] if wch == 2 else [_KT // wch] * wch)
        assert sum(WSPLIT) == _KT
        wofs = [sum(WSPLIT[:h]) for h in range(wch)]  # K-chunk offset per chunk

        xt = [io.tile([128, XW], mdt, name=f"xt{h}") for h in range(xch)]
        wt = [
            io.tile([128, 2 * _OSH * WSPLIT[h]], mdt, name=f"wt{h}")
            for h in range(wch)
        ]
        bct = io.tile([2, 2 * _OSH], mybir.dt.bfloat16)
        nc.sync.dma_start(out=xt[0][:], in_=xT[:, 0:XW])
        if xch > 1:
            nc.sync.dma_start(out=xt[1][:], in_=xT[:, XW : 2 * XW])
        for h in range(wch):
            c0 = 2 * _OSH * wofs[h]
            c1 = 2 * _OSH * (wofs[h] + WSPLIT[h])
            nc.sync.dma_start(out=wt[h][:], in_=wc[:, c0:c1])
        nc.sync.dma_start(out=bct[:], in_=bc[:])

        # single accumulation group: zc[:, 0:128] = z_non, zc[:, 128:256] = z_nmda
        # tiny bias rows accumulate last (their DMA is last in the stream)
        zc = ps.tile([_B, 2 * _OSH], dt)
        for j in range(_KT):
            xh, xj = divmod(j, XJ)
            wh = max(h for h in range(wch) if wofs[h] <= j)
            wj = j - wofs[wh]
            nc.tensor.matmul(
                zc[:], xt[xh][:, ts(xj, 128)], wt[wh][:, ts(wj, 2 * _OSH)],
                start=(j == 0), stop=False,
            )
        nc.tensor.matmul(zc[:], ones, bct[:], start=False, stop=True)
        zn = zc[:, 0:_OSH]
        zm = zc[:, _OSH : 2 * _OSH]

        # s = sigmoid(zn) = 1/(1+exp(-zn))
        e0 = ep.tile([_B, _OSH], dt)
        nc.scalar.activation(e0[:], zn, AF.Exp, scale=-1.0)
        d0 = ep.tile([_B, _OSH], dt)
        nc.vector.tensor_scalar_add(d0[:], e0[:], 1.0)
        s = ep.tile([_B, _OSH], dt)
        nc.vector.reciprocal_approx_fast(s[:], d0[:])
        # u = ln(2*relu(zm) + eps); relu+doubling on DVE (runs beside e0)
        ca2 = ep.tile([_B, _OSH], dt)
        nc.vector.tensor_scalar(ca2[:], zm, 0.0, 2.0, OP.max, OP.mult)
        u = ep.tile([_B, _OSH], dt)
        nc.scalar.activation(u[:], ca2[:], AF.Ln, bias=eps[:])
        # t = (s+1)*u = n_modif * u
        t = ep.tile([_B, _OSH], dt)
        nc.vector.scalar_tensor_tensor(t[:], s[:], 1.0, u[:], OP.add, OP.mult)
        # y = sigmoid(t)
        e1 = ep.tile([_B, _OSH], dt)
        nc.scalar.activation(e1[:], t[:], AF.Exp, scale=-1.0)
        d1 = ep.tile([_B, _OSH], dt)
        nc.vector.tensor_scalar_add(d1[:], e1[:], 1.0)
        y = ep.tile([_B, _OSH], dt)
        nc.vector.reciprocal_approx_fast(y[:], d1[:])
        # out = y + (s - 1)
        res = ep.tile([_B, _OSH], dt)
        nc.vector.scalar_tensor_tensor(res[:], y[:], -1.0, s[:], OP.add, OP.add)
        nc.sync.dma_start(out=out[:], in_=res[:])

    with tile.TileContext(nc) as tc:
        with (
            tc.tile_pool(name="io", bufs=1) as io,
            tc.tile_pool(name="ep", bufs=1) as ep,
            tc.tile_pool(name="ps", bufs=1, space="PSUM") as ps,
        ):
            if loop_n is None:
                body(tc, io, ep, ps)
            else:
                with tc.For_i(0, loop_n, 1):
                    body(tc, io, ep, ps)

    nc.compile()
    return nc, run_bass_kernel_spmd


def _prep_in_maps(inputs, W_nmda, W_non, b_non, mm_dtype=None):
    import ml_dtypes

    mm_dtype = mm_dtype or _MMDT
    npdt = np.float32 if mm_dtype in ("float32", "float32r") else ml_dtypes.bfloat16

    x = np.ascontiguousarray(np.asarray(inputs, dtype=np.float32))
    Wn = np.asarray(W_non, dtype=np.float32)
    Wm = np.asarray(W_nmda, dtype=np.float32).copy()
    Wm[:, 0] *= 0.5
    Wm[:, -1] *= 0.5
    b = np.asarray(b_non, dtype=np.float32)
    # bias applied as two bf16 K-rows: bh + bl reproduces b to ~2^-17
    bh = b.astype(ml_dtypes.bfloat16).astype(np.float32)
    bl = b - bh

    xr = _rearr(x).astype(npdt)
    in_maps = []
    for c in range(_NC):
        sl = slice(c * _OSH, (c + 1) * _OSH)
        wnr = _rearr(Wn[sl]).reshape(128, _KT, 128)
        wmr = _rearr(Wm[sl]).reshape(128, _KT, 128)
        wcat = np.ascontiguousarray(
            np.concatenate([wnr, wmr], axis=2).reshape(128, 2 * _IN)
        ).astype(npdt)
        bcat = np.zeros((2, 2 * _OSH), np.float32)
        bcat[0, :_OSH] = bh[sl]
        bcat[1, :_OSH] = bl[sl]
        in_maps.append(
            {"xT": xr, "wc": wcat, "bc": bcat.astype(ml_dtypes.bfloat16)}
        )
    return in_maps


def kernel(inputs, W_nmda, W_non, b_non):
    if "nc" not in _state:
        _state["nc"], _state["run"] = _build()
    nc, run = _state["nc"], _state["run"]
    in_maps = _prep_in_maps(inputs, W_nmda, W_non, b_non)
    res = run(nc, in_maps, list(range(_NC)))
    outs = res.results
    return np.concatenate([outs[c]["out"] for c in range(_NC)], axis=1)

